# revision 50
# baseline (speedup 1.0000x reference)
"""Trainium2 Bass kernel for DifferentiableLengthRegulator.

Math (per batch b):
  center = cumsum(w) - 0.5*w                          [T]
  y      = wsc2 * relu(pos - center)^2                [T, L]   wsc2 = 0.5*s*w^2
  W      = exp(-y); V = exp(W)                        [T, L]   (V in [1, e])
  P      = V / sum_T V        (softmax over already-exponentiated W)
  out    = (x*x_mask) @ P * y_mask                    [C, L]

Transposed-output design: out.T chunks [128 l-partitions, 256 c] so 1/den
is a per-partition scalar.  T rows are grouped into NT=5 tiles of ~103
rows (sorted by center; wide rows parked last); V tiles [row, l] built
only over each tile's band [lo128, hi) serve directly as matmul lhsT.
Spare partitions carry host-built "rider" rows: segment indicators (in V)
paired with per-segment pure-field vectors (in the x-side rhs), so pure
contributions and the den base ride inside the band matmuls.  den rides
as rhs column 256; reciprocal is a strided [128, 4] DVE op over PSUM.

V build via W = (sqrt(pi)/2) * DerivErf(sqrt(wsc2) * relu(l - c)) (the
2/sqrt(pi) factor cancels in softmax: V = exp(kappa*Wt)).  Three regions
per tile band: plateau (memset V_A), mixed (relu -> DerivErf -> exp),
decay (DerivErf straight off iota via per-row scale+bias -> exp).

Sharding: data-parallel over batch, 4 batches per core, 8 cores.
"""

import numpy as np

_B, _C, _T, _L = 32, 256, 512, 2048
_NC = 8
_BPC = _B // _NC
_NT = 5                   # T tiles per batch (rows sorted by center)
_NCH = _L // 128          # output l-chunks per batch
_TH = 8.0                 # y cutoff: beyond d=sqrt(TH/wsc2), V ~ V_C
_WIDE_CUT = 256.0         # rows with cut wider than this go to the back
_KSEG = 14                # max rider segments per (slot, tile)
_KAPPA = float(np.sqrt(np.pi) / 2)

LAST_RESULT = None

_OPS = {}


def _register(name, spec):
    import concourse.dve_ops as dops
    from concourse.dve_spec import lower
    from concourse.dve_ops import has_src1, DveOpSpec

    if name in _OPS:
        return _OPS[name]
    op = dops.DveOp(name, spec, subdim=False, uops_sha={})
    row = max(dops._SUB_OPCODE_FOR_NAME.values()) + 1
    assert row < 0x20, row
    dops.OPS.append(op)
    dops.CUSTOM_DVE_SPECS[op.name] = spec
    dops._SUB_OPCODE_FOR_NAME[op.name] = row
    for ver in ("v3", "v4"):
        s2 = DveOpSpec(name=op.name, opcode=row, uops=lower(spec, ver=ver),
                       rd1_en=has_src1(spec))
        op.uops_sha[ver] = s2.sha(ver)
    _OPS[name] = op
    return op


def _make_ops():
    from concourse.dve_spec import Spec, Src0, C0, C1, C2, sq, relu, minn

    # y = min(wsc2 * relu(l + (-c))^2, 8.0); s0=-c, s1=wsc2, imm2=8.0
    clamp5 = _register("CLAMP5_ANT", Spec(
        body=minn(sq(relu(Src0 + C0)) * C1, C2),
        reference=lambda in0, in1, s0, s1, imm2: np.float32(
            np.minimum(np.square(np.maximum(in0 + s0, 0.0)) * s1, imm2))))

    # q = (1 + u*C0)^64 (8 stages) -- exp(u) approx valve for V
    def lin6_ref(in0, in1, s0, s1, imm2):
        q = np.float32(1.0) + in0 * np.float32(s0)
        for _ in range(6):
            q = q * q
        return np.float32(q)

    from concourse.dve_spec import One
    p = One + Src0 * C0
    for _ in range(6):
        p = sq(p)
    lin6 = _register("LIN6_ANT", Spec(body=p, reference=lin6_ref))
    return clamp5, lin6


_C0_V = 0.0157265625      # lin6 constant for exp(u), u in [0,1]


def _lin6_np(u, c0):
    q = 1.0 + np.asarray(u, np.float64) * c0
    for _ in range(6):
        q = q * q
    return q


def _install_trace_shim():
    import sys
    import types

    try:
        from antenv.axon_hooks import get_axon_ntff_profile_hook  # noqa: F401
        return
    except ImportError:
        pass
    from trn_agent_boot.trn_boot import _ntff_profile_via_ctypes

    hook = _ntff_profile_via_ctypes("/opt/axon/libaxon_pjrt.so")
    mod = types.ModuleType("antenv.axon_hooks")
    mod.get_axon_ntff_profile_hook = lambda: hook
    mod.set_axon_ntff_profile_hook = lambda h: None
    sys.modules["antenv.axon_hooks"] = mod

    import concourse.bass_utils as bu

    bu.upload_artifacts = lambda tmpdir: f"local://{tmpdir}"


def _build_and_run(xtd, ccol, wcol, sqcol, bcol, iota_row, vrid, plan,
                   ym_trivial, ymcol, trace=False, tmpdir=None):
    from contextlib import ExitStack

    import concourse.bass as bass
    import concourse.tile as tile
    from concourse import bacc, mybir
    from concourse.bass_utils import run_bass_kernel_spmd

    f32 = mybir.dt.float32
    f16 = mybir.dt.float16
    Alu = mybir.AluOpType
    Act = mybir.ActivationFunctionType

    clamp5_op, lin6_op = _make_ops()
    nc = bacc.Bacc("TRN2", target_bir_lowering=False, debug=False,
                   num_devices=_NC)
    xtd_d = nc.dram_tensor("xtd", [_BPC, _NT, 128, 257], f16,
                           kind="ExternalInput")
    ccol_d = nc.dram_tensor("ccol", [128, _BPC * _NT], f32,
                            kind="ExternalInput")
    wcol_d = nc.dram_tensor("wcol", [128, _BPC * _NT], f32,
                            kind="ExternalInput")
    sqcol_d = nc.dram_tensor("sqcol", [128, _BPC * _NT], f32,
                             kind="ExternalInput")
    bcol_d = nc.dram_tensor("bcol", [128, _BPC * _NT], f32,
                            kind="ExternalInput")
    iota_d = nc.dram_tensor("iota", [1, _L], f16, kind="ExternalInput")
    vrid_d = nc.dram_tensor("vrid", [_BPC, _NT, _KSEG, _L], f16,
                            kind="ExternalInput")
    ym_d = nc.dram_tensor("ymc", [128, _BPC * _NCH], f32,
                          kind="ExternalInput")
    out_d = nc.dram_tensor("out", [_BPC, _NCH, 128, 256], f16,
                           kind="ExternalOutput")

    with tile.TileContext(nc) as tc, ExitStack() as ctx:
        singles = ctx.enter_context(tc.tile_pool(name="singles", bufs=1))
        xt_pool = ctx.enter_context(tc.tile_pool(name="xt", bufs=2))
        vv_pool = ctx.enter_context(tc.tile_pool(name="vv", bufs=2 * _NT))
        sc_pool = ctx.enter_context(tc.tile_pool(name="scp", bufs=4))
        rb_pool = ctx.enter_context(tc.tile_pool(name="rb", bufs=4))
        ob_pool = ctx.enter_context(tc.tile_pool(name="ob", bufs=4))
        pnum = ctx.enter_context(tc.tile_pool(name="pnum", bufs=2,
                                              space="PSUM"))

        iota_t = singles.tile([128, _L], f16)
        nc.sync.dma_start(out=iota_t[:], in_=bass.AP(
            tensor=iota_d, offset=0, ap=[[0, 128], [1, _L]]))
        ccol_t = singles.tile([128, _BPC * _NT], f32)
        nc.sync.dma_start(out=ccol_t[:], in_=ccol_d[:])
        wcol_t = singles.tile([128, _BPC * _NT], f32)
        nc.sync.dma_start(out=wcol_t[:], in_=wcol_d[:])
        sqcol_t = singles.tile([128, _BPC * _NT], f32)
        nc.sync.dma_start(out=sqcol_t[:], in_=sqcol_d[:])
        bcol_t = singles.tile([128, _BPC * _NT], f32)
        nc.sync.dma_start(out=bcol_t[:], in_=bcol_d[:])
        if not ym_trivial:
            ym_t = singles.tile([128, _BPC * _NCH], f32)
            nc.sync.dma_start(out=ym_t[:], in_=ym_d[:])

        # plateau V per V-engine (0=ACT exp exact, 1=DVE lin6)
        V_A_ENG = [float(np.float16(np.exp(1.0))),
                   float(np.float16(_lin6_np(1.0, _C0_V)))]

        def load_x(bb):
            xt = xt_pool.tile([128, _NT, 257], f16, tag="xt", name="xt")
            nc.sync.dma_start(out=xt[:], in_=bass.AP(
                tensor=xtd_d, offset=bb * _NT * 128 * 257,
                ap=[[257, 128], [128 * 257, _NT], [1, 257]]))
            return xt

        def build_tile_ops(bb, ti):
            """Return (vt, [thunks]) emitting the V build in ~512-col units."""
            P = plan[bb]
            lo, hi = P["bounds"][ti]
            pL, pR = P["regions"][ti]
            nb = hi - lo
            bt = bb * _NT + ti
            vt = vv_pool.tile([128, nb], f16, tag=f"vt{ti}", name=f"v{ti}")
            step = 384 if bb == 0 else 640
            veng = P["veng"][ti]
            thunks = []
            if pL > lo:
                def t_pl(a=lo, b=pL):
                    nc.vector.memset(vt[:, a - lo:b - lo], V_A_ENG[veng])
                thunks.append(t_pl)
            # mixed region: DVE clamp5 (y) -> ACT exp (W) -> exp/lin6 (V)
            sc = sc_pool.tile([128, max(nb, 8)], f16, tag=f"sc{ti % 2}",
                              name="sc")

            def emit_wv(a, b, wscale):
                nc.scalar.activation(
                    out=sc[:, a - lo:b - lo], in_=sc[:, a - lo:b - lo],
                    func=Act.Exp, scale=wscale)
                if veng == 0:
                    nc.scalar.activation(
                        out=vt[:, a - lo:b - lo], in_=sc[:, a - lo:b - lo],
                        func=Act.Exp)
                else:
                    nc.vector._custom_dve(
                        lin6_op, out=vt[:, a - lo:b - lo],
                        in0=sc[:, a - lo:b - lo], s0=_C0_V)

            a = pL
            while a < pR:
                b = min(a + step, pR)

                def t_mx(a=a, b=b):
                    # ccol holds -center
                    nc.vector._custom_dve(
                        clamp5_op, out=sc[:, a - lo:b - lo],
                        in0=iota_t[:, a:b], s0=ccol_t[:, bt:bt + 1],
                        s1=wcol_t[:, bt:bt + 1], imm2=_TH)
                    emit_wv(a, b, -1.0)
                thunks.append(t_mx)
                a = b
            # decay region (all rows right of center): ACT square off iota,
            # scaled by 1/256 to stay in f16 range
            a = pR
            while a < hi:
                b = min(a + step, hi)

                def t_dc(a=a, b=b):
                    nc.scalar.activation(
                        out=sc[:, a - lo:b - lo], in_=iota_t[:, a:b],
                        func=Act.Square, scale=sqcol_t[:, bt:bt + 1],
                        bias=bcol_t[:, bt:bt + 1])
                    emit_wv(a, b, -65536.0)
                thunks.append(t_dc)
                a = b
            r0, nrid = P["riders"][ti]
            if nrid > 0:
                def t_rid():
                    nc.sync.dma_start(
                        out=vt[r0:r0 + nrid, :],
                        in_=bass.AP(tensor=vrid_d,
                                    offset=((bb * _NT + ti) * _KSEG) * _L + lo,
                                    ap=[[_L, nrid], [1, nb]]))
                thunks.append(t_rid)
            return vt, thunks

        def main_group(bb, g, xt, vts, feng):
            P = plan[bb]
            pn = pnum.tile([128, 2048], f32, tag="pn", name=f"pn{g}")
            for kk in range(4):
                k = 4 * g + kk
                cov = P["cover"][k]
                for j, (ti, m1) in enumerate(cov):
                    lo, hi = P["bounds"][ti]
                    c0 = 128 * k - lo
                    nc.tensor.matmul(
                        pn[0:m1, kk * 512:kk * 512 + 257],
                        vts[ti][:, c0:c0 + m1],
                        xt[:, ti, :],
                        start=(j == 0), stop=(j == len(cov) - 1))
            rb = rb_pool.tile([128, 4], f32, tag="rb", name=f"rb{g}")
            den_ap = bass.AP(tensor=pn.tensor, offset=pn.offset + 256,
                             ap=[pn.ap[0], [512, 4]])
            nc.vector.reciprocal_approx_fast(out=rb[:], in_=den_ap)
            if not ym_trivial:
                nc.vector.tensor_tensor(
                    out=rb[:], in0=rb[:],
                    in1=ym_t[:, bb * _NCH + 4 * g:bb * _NCH + 4 * g + 4],
                    op=Alu.mult)
            ob = ob_pool.tile([128, 4, 256], f16, tag="ob", name=f"ob{g}")
            if feng == 0:
                pn_pages = bass.AP(tensor=pn.tensor, offset=pn.offset,
                                   ap=[pn.ap[0], [512, 4], [1, 256]])
                rb_b = bass.AP(tensor=rb.tensor, offset=rb.offset,
                               ap=[rb.ap[0], [1, 4], [0, 256]])
                nc.vector.tensor_tensor(out=ob[:], in0=pn_pages, in1=rb_b,
                                        op=Alu.mult)
            else:
                for kk in range(4):
                    nc.scalar.activation(
                        out=ob[:, kk, :], in_=pn[:, kk * 512:kk * 512 + 256],
                        func=Act.Copy, scale=rb[:, kk:kk + 1])
            nc.sync.dma_start(out=bass.AP(
                tensor=out_d, offset=(bb * _NCH + 4 * g) * 128 * 256,
                ap=[[256, 128], [128 * 256, 4], [1, 256]]), in_=ob[:])

        # software pipeline with interleaved V-build emission
        xts = {0: load_x(0)}
        cur_v = {}
        cur_thunks = []
        for ti in range(_NT):
            vt, th = build_tile_ops(0, ti)
            cur_v[ti] = vt
            cur_thunks.extend(th)
        for t in cur_thunks:
            t()
        for bb in range(_BPC):
            nxt_thunks = []
            if bb + 1 < _BPC:
                xts[bb + 1] = load_x(bb + 1)
                nxt_v = {}
                order = sorted(range(_NT), key=lambda t: -(
                    plan[bb + 1]["bounds"][t][1]
                    - plan[bb + 1]["bounds"][t][0]))
                for ti in order:
                    vt, th = build_tile_ops(bb + 1, ti)
                    nxt_v[ti] = vt
                    nxt_thunks.extend(th)
            n_th = len(nxt_thunks)
            idx = 0
            for g in range(4):
                main_group(bb, g, xts[bb], cur_v,
                           feng=1 if (bb + g) % 4 == 3 else 0)
                take = (n_th * (g + 1)) // 4 - idx
                for t in nxt_thunks[idx:idx + take]:
                    t()
                idx += take
            xts.pop(bb)
            if bb + 1 < _BPC:
                cur_v = nxt_v

    nc.compile()

    in_maps = []
    for i in range(_NC):
        in_maps.append({
            "xtd": xtd[i], "ccol": ccol[i], "wcol": wcol[i],
            "sqcol": sqcol[i], "bcol": bcol[i], "iota": iota_row,
            "vrid": vrid[i], "ymc": ymcol[i],
        })
    kwargs = {}
    if trace:
        _install_trace_shim()
        if tmpdir is not None:
            kwargs["tmpdir"] = tmpdir
    return run_bass_kernel_spmd(nc, in_maps, list(range(_NC)), trace=trace,
                                **kwargs)


def kernel(x, w, x_mask, y_mask, sigma_scale, _trace=False, _tmpdir=None):
    global LAST_RESULT
    x = np.ascontiguousarray(np.asarray(x, dtype=np.float32))
    w_ = np.asarray(w, dtype=np.float32)
    xm = np.asarray(x_mask, dtype=np.float32).reshape(_B, _T)
    ym = np.asarray(y_mask, dtype=np.float32).reshape(_B, _L)
    s = float(np.asarray(sigma_scale, dtype=np.float64).reshape(-1)[0])

    center = np.cumsum(w_, axis=1, dtype=np.float32) - np.float32(0.5) * w_
    wsc2 = 0.5 * s * w_.astype(np.float64) ** 2
    with np.errstate(divide="ignore"):
        cut = np.where(wsc2 > 0, np.sqrt(_TH / np.maximum(wsc2, 1e-300)),
                       np.inf)

    # device plateau values (f16-faithful)
    V_A_exp = np.float64(np.float16(np.exp(1.0)))
    V_A_deg = np.float64(np.float16(_lin6_np(1.0, _C0_V)))
    # right tail: V in [1, 1+e^-TH]; midpoint
    V_C_mid = float(np.exp(0.5 * np.exp(-_TH)))

    perm = np.empty((_B, _T), np.int64)
    minpark = np.full(_B, float(_L), np.float64)
    for b in range(_B):
        wide = cut[b] > _WIDE_CUT
        perm[b] = np.argsort(wide, kind="stable")
        if wide.any():
            minpark[b] = center[b][wide].min()
    center_p = np.take_along_axis(center, perm, axis=1)
    cut_p = np.take_along_axis(cut, perm, axis=1)
    wsc2_p = np.take_along_axis(wsc2, perm, axis=1)
    xm_p = np.take_along_axis(xm, perm, axis=1)

    # batch -> (core, slot): group by min parked-center so only some slots
    # pay a wide tile-4 band
    order = np.argsort(minpark, kind="stable")
    assign = np.empty((_NC, _BPC), np.int64)
    for bb in range(_BPC):
        for i in range(_NC):
            assign[i, bb] = order[bb * _NC + i]

    base = _T // _NT
    sizes = [base + (1 if t < _T % _NT else 0) for t in range(_NT)]
    spans = []
    a = 0
    for t in range(_NT):
        spans.append((a, a + sizes[t]))
        a += sizes[t]

    plan = []
    for bb in range(_BPC):
        grp = [int(assign[i, bb]) for i in range(_NC)]
        bounds = []
        cmins = []
        cmaxs = []
        for ti in range(_NT):
            r0, r1 = spans[ti]
            cmin = float(center_p[grp][:, r0:r1].min())
            cmax = float(center_p[grp][:, r0:r1].max())
            hi_t = float(np.minimum(center_p[grp][:, r0:r1]
                                    + cut_p[grp][:, r0:r1], 1e18).max())
            lo = int(np.clip(np.floor(cmin + 1e-6) // 128 * 128, 0, _L - 128))
            hi = int(np.clip(np.ceil((hi_t + 1e-3) / 8) * 8, 0, _L))
            hi = max(hi, lo + 8)
            bounds.append([lo, hi])
            cmins.append(cmin)
            cmaxs.append(cmax)
        evs = sorted(range(_NT), key=lambda t: bounds[t][0])
        covered = 0
        for t in evs:
            if bounds[t][0] > covered:
                bounds[t][0] = covered // 128 * 128
            covered = max(covered, bounds[t][1])
        if covered < _L:
            bounds[evs[-1]][1] = _L

        regions = []
        for ti in range(_NT):
            lo, hi = bounds[ti]
            pL = int(np.clip(np.floor(cmins[ti] - 1e-3) // 8 * 8, lo, hi))
            if pL - lo < 192:
                pL = lo            # too small to bother with memset
            pR = int(np.clip(np.ceil((cmaxs[ti] + 1e-3) / 8) * 8 + 8,
                             pL, hi))
            regions.append((pL, pR))

        cover = []
        for k in range(_NCH):
            cc = []
            for ti in range(_NT):
                lo, hi = bounds[ti]
                if lo <= 128 * k and hi > 128 * k:
                    cc.append((ti, min(hi - 128 * k, 128)))
            cc.sort(key=lambda t: -t[1])
            assert cc and cc[0][1] == 128, (bb, k, cc)
            cover.append(cc)

        bps = sorted({0, _L} | {b[0] for b in bounds} | {b[1] for b in bounds})
        segs = []
        for j in range(len(bps) - 1):
            sa, sb = bps[j], bps[j + 1]
            if sa >= sb:
                continue
            state = []
            for ti in range(_NT):
                lo, hi = bounds[ti]
                if sb <= lo:
                    state.append("A")
                elif sa >= hi:
                    state.append("C")
                else:
                    state.append("B")
            if all(st == "B" for st in state):
                continue
            host_ti = next(ti for ti in range(_NT) if state[ti] == "B")
            segs.append({"a": sa, "b": sb, "state": state, "host": host_ti})

        nrows = [spans[t][1] - spans[t][0] for t in range(_NT)]
        rid_cnt = [0] * _NT
        for sg in segs:
            sg["row"] = nrows[sg["host"]] + rid_cnt[sg["host"]]
            rid_cnt[sg["host"]] += 1
        assert all(nrows[t] + rid_cnt[t] <= 128 for t in range(_NT)), rid_cnt
        assert all(c <= _KSEG for c in rid_cnt), rid_cnt
        riders = [(nrows[t], rid_cnt[t]) for t in range(_NT)]

        # V engine per tile: widest tile's V on DVE lin6, rest ACT exp
        widths = [bounds[t][1] - bounds[t][0] for t in range(_NT)]
        veng = [0] * _NT
        veng[int(np.argmax(widths))] = 1
        plan.append({"bounds": [tuple(b) for b in bounds], "cover": cover,
                     "segs": segs, "riders": riders, "veng": veng,
                     "regions": regions, "spans": spans})

    xtd = np.zeros((_NC, _BPC, _NT, 128, 257), np.float16)
    ccol = np.full((_NC, 128, _BPC * _NT), -4096.0, np.float32)  # -center
    wcol = np.zeros((_NC, 128, _BPC * _NT), np.float32)          # wsc2
    sqcol = np.zeros((_NC, 128, _BPC * _NT), np.float32)         # sqrt/256
    bcol = np.zeros((_NC, 128, _BPC * _NT), np.float32)
    vrid = np.zeros((_NC, _BPC, _NT, _KSEG, _L), np.float16)
    ymcol = np.ones((_NC, 128, _BPC * _NCH), np.float32)
    ym_trivial = bool(np.all(ym == 1.0))

    for i in range(_NC):
        for bb in range(_BPC):
            b = int(assign[i, bb])
            P = plan[bb]
            xt_b = (x[b] * xm[b][None, :])[:, perm[b]]
            sq_b = np.sqrt(wsc2_p[b])
            for ti in range(_NT):
                r0, r1 = P["spans"][ti]
                n = r1 - r0
                bt = bb * _NT + ti
                xtd[i, bb, ti, :n, :256] = xt_b[:, r0:r1].T
                xtd[i, bb, ti, :n, 256] = xm_p[b, r0:r1]
                ccol[i, :n, bt] = -center_p[b, r0:r1]
                wcol[i, :n, bt] = wsc2_p[b, r0:r1]
                sqcol[i, :n, bt] = sq_b[r0:r1] / 256.0
                bcol[i, :n, bt] = -center_p[b, r0:r1] * sq_b[r0:r1] / 256.0
            xs = {}
            ms = {}
            for ti in range(_NT):
                r0, r1 = P["spans"][ti]
                xs[ti] = xt_b[:, r0:r1].astype(np.float64).sum(axis=1)
                ms[ti] = float(xm_p[b, r0:r1].sum())
            for sg in P["segs"]:
                ti = sg["host"]
                row = sg["row"]
                u = np.zeros(257, np.float64)
                for tj in range(_NT):
                    va = V_A_deg if P["veng"][tj] else V_A_exp
                    vc = V_C_mid
                    if sg["state"][tj] == "A":
                        u[:256] += va * xs[tj]
                        u[256] += va * ms[tj]
                    elif sg["state"][tj] == "C":
                        u[:256] += vc * xs[tj]
                        u[256] += vc * ms[tj]
                xtd[i, bb, ti, row, :] = u
                vrid[i, bb, ti, row - P["riders"][ti][0],
                     sg["a"]:sg["b"]] = 1.0
            if not ym_trivial:
                ymcol[i, :, bb * _NCH:(bb + 1) * _NCH] = \
                    ym[b].reshape(_NCH, 128).T

    iota_row = np.arange(_L, dtype=np.float16).reshape(1, _L)
    res = _build_and_run(xtd, ccol, wcol, sqcol, bcol, iota_row, vrid, plan,
                         ym_trivial, ymcol, trace=_trace, tmpdir=_tmpdir)
    LAST_RESULT = res

    out = np.empty((_B, _C, _L), np.float32)
    for i in range(_NC):
        for bb in range(_BPC):
            b = int(assign[i, bb])
            o = res.results[i]["out"][bb].reshape(_L, _C).astype(np.float32)
            out[b] = o.T
    return out
